# revision 1
# baseline (speedup 1.0000x reference)
"""Paged causal GQA prefill attention on 8 TRN2 NeuronCores.

Problem: B=4 seqs x S=1024 tokens, HQ=32 query heads, HK=8 KV heads, D=128,
paged KV cache (16 blocks x 256), causal, softmax scale 1/sqrt(128).

Sharding: tensor-parallel over heads. Core c owns KV head c and the G=4
query heads [4c, 4c+4) for all 4 sequences -> 16 (seq, head) units per core,
perfectly balanced, no collectives (output is disjoint across cores).

Per-unit algorithm (S^T layout, bf16 matmuls, f32 accumulation):
  S^T[k,q] = K^T.T @ Q^T   (lhsT = K^T[d,k] tile, rhs = Q^T[d,q], PSUM f32)
  P^T[k,q] = exp(SCALE * S^T)    (ScalarE ACTIVATE, PSUM->SBUF, bf16;
             3 ACTs/head over 1536-col PSUM score groups)
  diag blocks: zero k>q half     (GPSIMD affine_select, chunk-PAIRED:
             one strided [128,2,128] call per two diag blocks)
  O[q, 0:129] = sum_j P^T_j.T @ [V_j | 1]  (PSUM accumulate over k chunks;
               col 128 is the softmax denominator, no separate reduction)
  host divides numerator by denominator column.

Engine budget per head (measured, full clock): ScalarE exp 3x(1536+172)cyc
/1.2GHz = 4.3us -> 69us total, PE (QK 4608 cols + 36 PV matmuls) ~4.4us ->
~71us, DVE (3 o-PSUM casts) ~1.6us, GPSIMD (4 paired masks) ~1.7us.
ScalarE and PE are co-critical; ScalarE runs gapless in steady state and
paces the kernel. Every attempt to offload exp to the DVE (Schraudolph
int16-bitcast approx — machinery still here behind DVE_L) measured SLOWER;
see the DVE_L comment.

Score PSUM: 3 group tiles/head (1536 f32 = 3 banks) rotating 2 slots
(6 banks) + 2x 1-bank o tiles = all 8 banks. Groups {0,4} {1,3} {2,5,6,7}.

Pipeline: one-head software stagger — PE runs QK(n) interleaved with
PV(n-1) packs; ScalarE exps trail QK by one group; q tiles prefetch ONE
HEAD AHEAD (the in-order PE queue head-of-line blocks on the q transfer if
it is triggered in the consuming iteration); K/V prefetch one head before
a sequence boundary.

I/O: output DRAM layout is [B, G, 128, NT, D+1] (partition-major,
identical to the SBUF ob tile) so the store is one fully-contiguous
2064B-per-partition DMA per head (vs 258B strided runs = 16K extra DMA
packets with a token-major layout); the host re-permutes. DMA triggers are
merged (the SP queue was 70% busy on ~650ns-per-call descriptor
generation with split triggers), and the two transfers gating the first
matmul issue from the otherwise-idle ScalarE HWDGE queue.

Startup: dummy FD-128 matmuls (no data deps) fill the ~3.5us DMA-wait
window so the PE HAM clock-gate (K=4/8 until ~3.4us of sustained
activity) flips before real matmuls issue — without them the first ~2
heads run at half clock. Head-0 K is transferred in three pieces so each
QK group unblocks as its columns land.

Timing note: the device clock is noisy run-to-run (~20% downclock
episodes; compare runs via the ACT_TABLE_LOAD canary: 1283ns = full
clock, ~1539ns = downclocked). Best measured at full clock: 87.7us
(baseline: 91.1us; all numbers in comments are full-clock equivalents).
"""

import numpy as np
import ml_dtypes
import math as _math
from contextlib import ExitStack

import concourse.bass as bass
import concourse.tile as tile
from concourse import bacc, mybir
from concourse.bass_utils import run_bass_kernel_spmd

B, S, HQ, HK, D = 4, 1024, 32, 8, 128
BS = 256
G = HQ // HK            # 4 query heads per KV head
NCORES = 8
NT = S // 128           # 8 key chunks / query tiles of 128
SCALE = 1.0 / float(np.sqrt(D))

BF16 = mybir.dt.bfloat16
F32 = mybir.dt.float32
I16 = mybir.dt.int16
_BF16_NP = ml_dtypes.bfloat16

EXP_A = SCALE * 128.0 / _math.log(2.0)  # fold softmax scale into the fma
EXP_B = 128.0 * 127.0 - 6.0             # bias, c=6 calibrated for min rms

# chunk j covers keys [128j, 128(j+1)) and queries q in [128j, S)
CHUNK_W = {j: S - 128 * j for j in range(NT)}

# chunks packed in the P^T tile (bf16 pt for ScalarE heads, int16 d for
# DVE heads — same column layout), group-major
SC_OFF = {0: 0, 4: 1024, 1: 1536, 3: 2432, 2: 3072, 5: 3840, 6: 4224, 7: 4480}
PT_COLS = 4608
# +128 pad: the strided pair-mask view of the last chunk pair (c6@4224,
# c7@4480, stride 256) spans cols up to 4736 even though only :128 of each
# 256-block is touched
PT_ALLOC = PT_COLS + 128

# score psum groups: (chunk, psum col offset) per group
GROUPS = [
    [(0, 0), (4, 1024)],
    [(1, 0), (3, 896)],
    [(2, 0), (5, 768), (6, 1152), (7, 1408)],
]
SLOT_W = 1536
# PV accumulation order: chunks ranked by which ACT produces their exp
# (group 0 first, group 2 last)
PV_RANK = {0: 0, 4: 1, 1: 2, 3: 3, 2: 4, 5: 5, 6: 6, 7: 7}

# Heads with query-head index l == DVE_L run their ENTIRE exp on the DVE
# (Schraudolph) instead of ScalarE. Every offload granularity was measured
# SLOWER end-to-end than all-ScalarE exp, despite ScalarE being the
# busiest engine (~80us) and DVE the idlest (~35us): chunk-granular splits
# cost ~12us (each PSUM score group gains a second consumer engine +
# ~0.4-0.8us cross-engine sem latency per rendezvous); whole-head splits
# (single-consumer groups) still cost ~8us even with the G2 exp split to
# unblock the first PV pack — the DVE TS+mask chain latency stalls the PV
# matmuls that consume P^T. Disabled (None) = all exp on ScalarE.
DVE_L = None

_NC_CACHE = None


def _emit(tc, qT, kT, vp, out):
    nc = tc.nc
    Exp = mybir.ActivationFunctionType.Exp

    with ExitStack() as ctx:
        kv_pool = ctx.enter_context(tc.tile_pool(name="kv", bufs=3))
        q_pool = ctx.enter_context(tc.tile_pool(name="q", bufs=4))
        pt_pool = ctx.enter_context(tc.tile_pool(name="pt", bufs=3))
        dve_pool = ctx.enter_context(tc.tile_pool(name="dve", bufs=3))
        s_psum = ctx.enter_context(tc.tile_pool(name="s_psum", bufs=2, space="PSUM"))
        o_psum = ctx.enter_context(tc.tile_pool(name="o_psum", bufs=2, space="PSUM"))
        ob_pool = ctx.enter_context(tc.tile_pool(name="ob", bufs=6))
        singles = ctx.enter_context(tc.tile_pool(name="singles", bufs=1))

        # HAM warm-up: the PE clock gate starts at K=4/8 (1.2 GHz) and
        # needs ~3.4us of sustained matmul activity to flip to 8/8. The
        # first real matmul cannot issue until its DMAs land (~10us), and
        # without priming, the first ~2 heads of matmuls run at half
        # clock. These dummy FD-128 matmuls have no data dependencies, so
        # they fill the DMA-wait window and end roughly when data lands.
        wk = singles.tile([128, 128], BF16)
        nc.vector.memset(wk, 0.0)
        w_ps = o_psum.tile([128, 3, D + 1], F32, tag="o")
        for _ in range(32):
            nc.tensor.matmul(
                w_ps[:, 0, :128], lhsT=wk, rhs=wk, start=True, stop=True
            )

        heads = [(b, l) for b in range(B) for l in range(G)]
        stage = {}
        kv_cur = None

        def load_kv(bb):
            kt_t = kv_pool.tile([D, S], BF16, tag="kt")
            nc.sync.dma_start(out=kt_t[:, :128], in_=kT[bb][:, :128])
            nc.sync.dma_start(out=kt_t[:, 128:], in_=kT[bb][:, 128:])
            vp_t = kv_pool.tile([128, NT, D + 1], BF16, tag="vp")
            nc.sync.dma_start(out=vp_t, in_=vp[bb])
            return kt_t, vp_t

        # Software pipeline staggered by one head: PE runs QK^T(n) while
        # ScalarE/DVE exp head n-1..n scores; PV(n-1) P^T is ready by then.
        # q tiles are prefetched ONE HEAD AHEAD: the in-order PE queue
        # head-of-line blocks on the q transfer if it is triggered in the
        # same iteration that consumes it.
        q_tiles = {}
        for n in range(len(heads) + 1):
            if n + 1 < len(heads) and n >= 1:
                nb, nl = heads[n + 1]
                qn = q_pool.tile([D, S], BF16, tag="q")
                nc.sync.dma_start(out=qn, in_=qT[nb, nl])
                q_tiles[n + 1] = qn
            if n < len(heads):
                b, l = heads[n]
                if n == 0:
                    # the two transfers gating the first QK matmul go on the
                    # ScalarE HWDGE queue (idle until its first ACT) so they
                    # are not serialized behind the SP queue's ~650ns/call
                    # descriptor generation for the other head-0 transfers
                    kt0 = kv_pool.tile([D, S], BF16, tag="kt")
                    nc.scalar.dma_start(out=kt0[:, :128], in_=kT[0][:, :128])
                    q_t = q_pool.tile([D, S], BF16, tag="q")
                    nc.scalar.dma_start(out=q_t[:, :512], in_=qT[b, l][:, :512])
                    nc.sync.dma_start(out=q_t[:, 512:], in_=qT[b, l][:, 512:])
                    nc.sync.dma_start(out=kt0[:, 128:512], in_=kT[0][:, 128:512])
                    nc.sync.dma_start(out=kt0[:, 512:], in_=kT[0][:, 512:])
                    q1 = q_pool.tile([D, S], BF16, tag="q")
                    nc.sync.dma_start(out=q1, in_=qT[heads[1][0], heads[1][1]])
                    q_tiles[1] = q1
                    # vp0 (1MB, not consumed until iteration 1's PV) goes
                    # LAST so its transfer does not contend for DMA-engine
                    # bandwidth with the head-0/1 QK-critical pieces
                    vp0 = kv_pool.tile([128, NT, D + 1], BF16, tag="vp")
                    nc.sync.dma_start(out=vp0, in_=vp[0])
                    kv_next = (kt0, vp0)
                    # trigger the exp ACT_TABLE_LOAD (~2.7us) behind the
                    # startup triggers, still overlapping the DMAs
                    warm = singles.tile([1, 1], F32)
                    nc.vector.memset(warm, 0.0)
                    nc.scalar.activation(out=warm, in_=warm, func=Exp)
                else:
                    q_t = q_tiles.pop(n)
                if l == 0:
                    kv_cur, kv_next = kv_next, None
                if l == G - 1 and b + 1 < B:
                    # prefetch the next sequence's K/V one head early
                    kv_next = load_kv(b + 1)
                kt_t, vp_t = kv_cur

                is_dve = l == DVE_L
                if is_dve:
                    pt_t = dve_pool.tile([128, PT_ALLOC], I16, tag="d")
                else:
                    pt_t = pt_pool.tile([128, PT_ALLOC], BF16, tag="pt")

                def diag_masks(g, pt_t=pt_t):
                    # zero the strictly-upper half (k > q, i.e. free idx c <
                    # partition idx p) of each chunk's diagonal 128x128
                    # block. Chunks are packed adjacently per group, so two
                    # diag blocks are one strided [128, 2, 128] view -> one
                    # affine_select per chunk PAIR (halves the GPSIMD call +
                    # semaphore count on the ACT->mask->PV latency chain).
                    k = 0
                    while k < len(g):
                        j0 = g[k][0]
                        base = SC_OFF[j0]
                        if k + 1 < len(g):
                            stride = SC_OFF[g[k + 1][0]] - base
                            dg = pt_t[:, base : base + 2 * stride].rearrange(
                                "p (g c) -> p g c", g=2
                            )[:, :, :128]
                            pattern = [[0, 2], [1, 128]]
                            k += 2
                        else:
                            dg = pt_t[:, base : base + 128]
                            pattern = [[1, 128]]
                            k += 1
                        nc.gpsimd.affine_select(
                            out=dg,
                            in_=dg,
                            pattern=pattern,
                            compare_op=mybir.AluOpType.is_ge,
                            fill=0.0,
                            base=0,
                            channel_multiplier=-1,
                        )

                def qk_group(gi, kt_t=kt_t, q_t=q_t, pt_t=pt_t, is_dve=is_dve):
                    g = GROUPS[gi]
                    s_t = s_psum.tile([128, SLOT_W], F32, tag="s")

                    def mms(j, local):
                        ext = CHUNK_W[j]
                        # segment matmuls, never crossing a 512-col PSUM bank
                        q0 = 0
                        while q0 < ext:
                            lo = local + q0
                            w = min(512 - (lo % 512), ext - q0)
                            nc.tensor.matmul(
                                s_t[:, lo : lo + w],
                                lhsT=kt_t[:, 128 * j : 128 * (j + 1)],
                                rhs=q_t[:, 128 * j + q0 : 128 * j + q0 + w],
                                start=True,
                                stop=True,
                            )
                            q0 += w

                    for j, local in g:
                        mms(j, local)

                    def exp_part(gpart, s_t=s_t):
                        lo = gpart[0][1]
                        w = sum(CHUNK_W[j] for j, _ in gpart)
                        base = SC_OFF[gpart[0][0]]
                        if is_dve:
                            nc.vector.tensor_scalar(
                                out=pt_t[:, base : base + w],
                                in0=s_t[:, lo : lo + w],
                                scalar1=EXP_A,
                                scalar2=EXP_B,
                                op0=mybir.AluOpType.mult,
                                op1=mybir.AluOpType.add,
                            )
                        else:
                            nc.scalar.activation(
                                out=pt_t[:, base : base + w],
                                in_=s_t[:, lo : lo + w],
                                func=Exp,
                                scale=SCALE,
                            )
                        diag_masks(gpart)

                    if not is_dve:
                        exp_part(g)
                        return None
                    # DVE heads: exp is deferred until after the next PV
                    # pack's cast, so the cast (which gates o-PSUM recycling
                    # for the PE) is never queued behind exp work in DVE's
                    # in-order queue. Exception: chunk 2 feeds the FIRST PV
                    # pack of this head next iteration — its exp fires
                    # immediately after its matmuls or that pack stalls.
                    if gi == 2:
                        exp_part(g[:1])
                        return lambda: exp_part(g[1:])
                    return lambda: exp_part(g)

                tail0 = qk_group(0)
                stage[n] = (pt_t, is_dve, vp_t, b, l, qk_group)

            def pv_pack(i_lo, i_hi, st):
                # 3 PV outputs share one PSUM bank; one wide cast per bank
                ppt_t, p_dve, pvp_t, pb, pl, _ = st

                def lhsT(i, j):
                    co = SC_OFF[j] + 128 * (i - j)
                    if p_dve:
                        return ppt_t[:, co : co + 128].bitcast(BF16)
                    return ppt_t[:, co : co + 128]

                o_t = o_psum.tile([128, i_hi - i_lo, D + 1], F32, tag="o")
                for i in range(i_lo, i_hi):
                    # accumulate chunks in exp-readiness order (group G0's
                    # chunks first, G2's last): the in-order PE queue would
                    # otherwise head-of-line block on a chunk whose ACT+mask
                    # lands late, stalling every matmul emitted behind it
                    # (worth ~1us on the final head's drain)
                    js = sorted(range(i + 1), key=lambda j: PV_RANK[j])
                    for j in js:
                        nc.tensor.matmul(
                            o_t[:, i - i_lo, :],
                            lhsT=lhsT(i, j),
                            rhs=pvp_t[:, j, :],
                            start=(j == js[0]),
                            stop=(j == js[-1]),
                        )
                # unnormalized numerator + denominator column; the
                # softmax divide happens on the host
                nc.vector.tensor_copy(ob_t[:, i_lo:i_hi, :], o_t)
                if n == len(heads):
                    # final head: store each pack as soon as it is cast, on
                    # the ScalarE HWDGE queue — idle after the last ACT, so
                    # the ~650ns/call descriptor generation runs in parallel
                    # with the SP queue instead of serializing the drain
                    nc.scalar.dma_start(
                        out=out[st[3], st[4], :, i_lo:i_hi],
                        in_=ob_t[:, i_lo:i_hi],
                    )

            if n > 0:
                prev = stage.pop(n - 1)
                ob_t = ob_pool.tile([128, NT, D + 1], BF16, tag="ob")
                pv_pack(0, 3, prev)

            if n < len(heads):
                if tail0 is not None:
                    tail0()
                tail1 = stage[n][5](1)

            if n > 0:
                pv_pack(3, 6, prev)

            if n < len(heads):
                if tail1 is not None:
                    tail1()
                tail2 = stage[n][5](2)

            if n > 0:
                pv_pack(6, NT, prev)
                if n < len(heads):
                    nc.sync.dma_start(out=out[prev[3], prev[4]], in_=ob_t)

            if n < len(heads) and tail2 is not None:
                tail2()


def _build():
    nc = bacc.Bacc(
        "TRN2", target_bir_lowering=False, debug=False, enable_asserts=False
    )
    qT = nc.dram_tensor("qT", [B, G, D, S], BF16, kind="ExternalInput").ap()
    kT = nc.dram_tensor("kT", [B, D, S], BF16, kind="ExternalInput").ap()
    vp = nc.dram_tensor("vp", [B, 128, NT, D + 1], BF16, kind="ExternalInput").ap()
    out = nc.dram_tensor(
        "out", [B, G, 128, NT, D + 1], BF16, kind="ExternalOutput"
    ).ap()
    with tile.TileContext(nc) as tc:
        _emit(tc, qT, kT, vp, out)
    nc.compile()
    return nc


def get_nc():
    global _NC_CACHE
    if _NC_CACHE is None:
        _NC_CACHE = _build()
    return _NC_CACHE


def make_in_maps(q, k_cache, v_cache, block_table):
    q = np.asarray(q, dtype=np.float32)
    k_cache = np.asarray(k_cache, dtype=np.float32)
    v_cache = np.asarray(v_cache, dtype=np.float32)
    block_table = np.asarray(block_table)

    q_r = q.reshape(B, S, HQ, D)
    in_maps = []
    for c in range(NCORES):
        # [B, G, D, S] query, transposed to d-major
        qT_c = np.ascontiguousarray(
            q_r[:, :, G * c : G * (c + 1), :].transpose(0, 2, 3, 1)
        ).astype(_BF16_NP)
        kT_c = np.empty((B, D, S), dtype=_BF16_NP)
        # [B, 128, NT, D+1]: partition-major V' so device rows are contiguous
        vp_c = np.empty((B, 128, NT, D + 1), dtype=_BF16_NP)
        for b in range(B):
            blocks = block_table[b]  # logical -> physical page ids
            k_seq = k_cache[blocks, :, c, :].reshape(S, D)
            v_seq = v_cache[blocks, :, c, :].reshape(S, D)
            kT_c[b] = k_seq.T.astype(_BF16_NP)
            # token 128*j + p -> vp_c[b, p, j, :]
            vp_c[b, :, :, :D] = (
                v_seq.reshape(NT, 128, D).transpose(1, 0, 2).astype(_BF16_NP)
            )
            vp_c[b, :, :, D] = 1.0
        in_maps.append({"qT": qT_c, "kT": kT_c, "vp": vp_c})
    return in_maps


def assemble_out(results):
    full = np.empty((B, S, HQ, D), dtype=np.float32)
    for c in range(NCORES):
        o = np.asarray(results[c]["out"], dtype=np.float32)  # [B,G,128,NT,D+1]
        # (b, l, p, i, d) -> token 128*i + p
        o = o.transpose(0, 3, 2, 1, 4).reshape(B, S, G, D + 1)
        full[:, :, G * c : G * (c + 1), :] = o[..., :D] / o[..., D:]
    return full.reshape(B * S, HQ * D)


def kernel(q, k_cache, v_cache, block_table):
    nc = get_nc()
    in_maps = make_in_maps(q, k_cache, v_cache, block_table)
    res = run_bass_kernel_spmd(nc, in_maps, core_ids=list(range(NCORES)))
    return assemble_out(res.results)

